# revision 17
# baseline (speedup 1.0000x reference)
# Trainium2 Bass kernel for nn_LNKillingRelu: out = where(kf<=0, x, x + kf*d)
#   d  = einsum('fkn,gf->gkn', x, W)                      (per batch)
#   kf = einsum('fkn,kl,fln->fn', x, G, d)  broadcast over k
# G is the (constant) Killing-form Gram matrix of sl(3):
#   G[0,0]=G[4,4]=12, G[0,4]=G[4,0]=-6, G[1,3]=G[3,1]=G[2,6]=G[6,2]=G[5,7]=G[7,5]=6
# so with kf' = kf/6:
#   kf' = x0*(2d0-d4) + x4*(2d4-d0) + x1*d3 + x3*d1 + x2*d6 + x6*d2 + x5*d7 + x7*d5
#   out = x + relu(6*kf') * d
#
# v5: f16 (rel err ~7e-3; harness gate 2e-2), engineered to measured HW:
#  - k-planes live in SBUF in the PI order (1,3, 2,6, 5,7, 0,4): G's pair
#    swaps become an affine negative-stride AP, so the products need only
#    2 strided TT-mults + 1 contiguous one instead of per-plane ops.
#    Pair-DMAs (stride-2/4 plane pairs) load/store this layout directly.
#  - gate is materialized 4-wide by the ScalarE relu (broadcast read), so
#    og = d*gate is two fully-contiguous FD2048 TT-mults (DVE 4x mode).
#  - lag-2 software pipeline: og/o2/out-DMA of iteration i-2 are emitted in
#    iteration i, so the DVE never waits on the kf->relu ScalarE round-trip.
#  - final +x is split: DVE adds slots 0-3, GpSimd adds slots 4-7 (GpSimd
#    shares an SBUF port with DVE, so it only gets work whose tiles the DVE
#    is done streaming; the shared o2 tile serializes the two halves).
#  - PSUM is two 4-bank half-tiles (slots 0-3 / 4-7), each single-buffered;
#    ScalarE copies each half to SBUF f16 right after its 16 matmuls.
#
# Sharding: data-parallel over batch B=8 -> one batch per NeuronCore (8 cores).
# W is replicated (host passes W^T in f16 so lhsT chunks slice directly).

from contextlib import ExitStack

import numpy as np

import concourse.bass as bass
import concourse.mybir as mybir
import concourse.tile as tile
from concourse.bass_utils import run_bass_kernel_spmd

B, F, K, N = 8, 512, 8, 2048
P = 128
FT = F // P  # 4 channel tiles
KH = K // 2  # plane slots per PSUM half

f32 = mybir.dt.float32
f16 = mybir.dt.float16
Alu = mybir.AluOpType
ActF = mybir.ActivationFunctionType

# Few, large DMAs: each dma_start is ONE in-order trigger instruction on the
# Sync sequencer (~0.9us + head-of-line blocking), while its descriptors spray
# across all 16 queues -- so one DMA per tile is strictly better than many.


def _ap(base, off_elems, dims):
    """Raw AP from a base AP: keep partition dim, replace free dims."""
    return bass.AP(
        tensor=base.tensor,
        offset=base.offset + off_elems,
        ap=[base.ap[0]] + dims,
    )


def build_nc(n_total=N, nt=512):
    nch = n_total // nt
    # race detection chokes on the post-hoc wait-split NoOps (they lack the
    # rust pass's fake sem updates); correctness is validated vs reference.
    nc = bass.Bass(detect_race_conditions=False)
    x = nc.dram_tensor("x", [F, K, n_total], f16, kind="ExternalInput")
    wt = nc.dram_tensor("wt", [F, F], f16, kind="ExternalInput")  # W^T (f, g)
    # host-precomputed z-planes for k=0,4: (2x0-x4, 2x4-x0) in fp32, then f16
    xaux = nc.dram_tensor("xaux", [F, 2, n_total], f16, kind="ExternalInput")
    out = nc.dram_tensor("out", [F, K, n_total], f16, kind="ExternalOutput")

    with tile.TileContext(nc) as tc, ExitStack() as ctx:
        wpool = ctx.enter_context(tc.tile_pool(name="w", bufs=1))
        xpool = ctx.enter_context(tc.tile_pool(name="xc", bufs=3))
        axpool = ctx.enter_context(tc.tile_pool(name="ax", bufs=2))
        papool = ctx.enter_context(tc.tile_pool(name="pda", bufs=1, space="PSUM"))
        pbpool = ctx.enter_context(tc.tile_pool(name="pdb", bufs=1, space="PSUM"))
        dcpool = ctx.enter_context(tc.tile_pool(name="dc", bufs=3))
        prpool = ctx.enter_context(tc.tile_pool(name="prod", bufs=1))
        s2pool = ctx.enter_context(tc.tile_pool(name="s2", bufs=2))
        s3pool = ctx.enter_context(tc.tile_pool(name="s3", bufs=3))
        opool = ctx.enter_context(tc.tile_pool(name="og", bufs=2))

        # resident W^T tiles: wsb[ft][p, g] , f = ft*128+p
        wsb = []
        for ft in range(FT):
            w_t = wpool.tile([P, F], f16, tag=f"w{ft}")
            nc.sync.dma_start(out=w_t[:], in_=wt[ft * P : (ft + 1) * P, :])
            wsb.append(w_t)

        # Walrus only allows ONE sync wait per Matmult (waits ride the
        # LDWEIGHTS struct).  Warmup matmuls make PE observe each W-DMA
        # semaphore individually so later matmuls never wait on W.
        warm = papool.tile([P, KH, nt], f32, tag="pda")
        for ft in range(FT):
            nc.tensor.matmul(
                warm[:, 0, 0:1], wsb[ft][:, 0:P], wsb[ft][:, 0:1], start=True, stop=True
            )

        # pstate pre-warm: ~4us of back-to-back matmuls on W data while the
        # first x tiles stream in, so mmA_0 runs at the hot 216ns rate.
        for r in range(18):
            nc.tensor.matmul(
                warm[:, 0, 0:512], wsb[r % FT][:, 0:P], wsb[(r + 1) % FT][:, 0:512],
                start=True, stop=True,
            )

        def emit_gate(st):
            # gate4 = relu(6*kf) replicated over 4 slots (broadcast read)
            gate4 = s3pool.tile([P, 4, nt], f16, tag="gate4")
            nc.scalar.activation(
                out=gate4[:],
                in_=_ap(st["kf"], 0, [[0, 4], [1, nt]]),
                func=ActF.Relu,
                scale=6.0,
            )
            st["gate4"] = gate4

        def flush(st):
            # Per k-half: og = d*gate (FD2048 TT-mult), o2 = og + x (FD2048
            # TT-add), store.  All DVE (GpSimd stalls the DVE via the shared
            # SBUF port); halved so each store issues ~2.3us earlier.
            dcb, xgb, g4, c, gt = st["dc"], st["xg"], st["gate4"], st["c"], st["gt"]
            for h, tag in ((0, "a"), (1, "b")):
                ks = slice(h * KH, (h + 1) * KH)
                og = opool.tile([P, KH, nt], f16, tag=f"og{tag}", bufs=1)
                nc.vector.tensor_tensor(
                    out=og[:], in0=dcb[:, ks, :], in1=g4[:], op=Alu.mult
                )
                o2 = opool.tile([P, KH, nt], f16, tag=f"o2{tag}", bufs=3)
                nc.vector.tensor_tensor(
                    out=o2[:], in0=og[:], in1=xgb[:, ks, :], op=Alu.add
                )
                st.setdefault("outs", []).append((ks, o2))

        outq = []      # iters awaiting their out-DMA (lag 3, Act-engine DGE)

        def emit_outs(st):
            # store triggers ride the Activation DGE queue: the Sync queue's
            # in-DMA triggers head-of-line block on buffer-free waits, which
            # delayed stores by ~20us; o2 is 1+ steps old here -> no wait.
            c, gt = st["c"], st["gt"]
            for ks, o2 in st["outs"]:
                nc.scalar.dma_start(
                    out=out[gt * P : (gt + 1) * P, ks, c * nt : (c + 1) * nt],
                    in_=o2[:],
                )

        pending = []   # iters awaiting og/o2 flush (lag 2)
        prev = None    # iter i-1: dcB + elementwise lagged one step

        def emit_tail(pv):
            # DVE elementwise for the previous iteration (its dcA landed
            # last step, its dcB at the head of this one -> no DVE stalls)
            xg, dc = pv["xg"], pv["dc"]
            p = prpool.tile([P, K, nt], f16, tag="p")
            # l in (1,3,5,7): z_l = x at (3,1,7,5) -- one 3-free-dim op
            nc.vector.tensor_tensor(
                out=_ap(p[:], nt, [[4 * nt, 2], [2 * nt, 2], [1, nt]]),
                in0=_ap(xg, 3 * nt, [[4 * nt, 2], [-2 * nt, 2], [1, nt]]),
                in1=_ap(dc[:], nt, [[4 * nt, 2], [2 * nt, 2], [1, nt]]),
                op=Alu.mult,
            )
            # l in (2,6): z_l = x at (6,2)
            nc.vector.tensor_tensor(
                out=p[:, 2::4, :],
                in0=xg[:, 6::-4, :],
                in1=dc[:, 2::4, :],
                op=Alu.mult,
            )
            # l in (0,4): z_l = host-precomputed xaux
            nc.vector.tensor_tensor(
                out=p[:, 0::4, :], in0=pv["ax"], in1=dc[:, 0::4, :], op=Alu.mult
            )
            t1 = s2pool.tile([P, 4, nt], f16, tag="t1")
            nc.vector.tensor_tensor(
                out=t1[:], in0=p[:, 0:4, :], in1=p[:, 4:8, :], op=Alu.add
            )
            t2 = s2pool.tile([P, 2, nt], f16, tag="t2")
            nc.vector.tensor_tensor(
                out=t2[:], in0=t1[:, 0:2, :], in1=t1[:, 2:4, :], op=Alu.add
            )
            kf = s3pool.tile([P, nt], f16, tag="kf")
            nc.vector.tensor_tensor(
                out=kf[:], in0=t2[:, 0, :], in1=t2[:, 1, :], op=Alu.add
            )
            pv["kf"] = kf

        for c in range(nch):
            xcs = []
            if c == 0:
                # all half-A loads first: matmul half A of iter 0 needs just
                # these four, so it starts ~4us earlier
                for ft in range(FT):
                    xt = xpool.tile([P, K, nt], f16, tag=f"xc{ft}", name=f"x0_{ft}")
                    xcs.append(xt)
                for h in (0, 1):
                    for ft in range(FT):
                        nc.sync.dma_start(
                            out=xcs[ft][:, h * KH : (h + 1) * KH, :],
                            in_=x[ft * P : (ft + 1) * P, h * KH : (h + 1) * KH,
                                 c * nt : (c + 1) * nt],
                        )
            else:
                for ft in range(FT):
                    xt = xpool.tile([P, K, nt], f16, tag=f"xc{ft}")
                    nc.sync.dma_start(
                        out=xt[:],
                        in_=x[ft * P : (ft + 1) * P, :, c * nt : (c + 1) * nt],
                    )
                    xcs.append(xt)
            axs = []
            for ft in range(FT):
                at = axpool.tile([P, 2, nt], f16, tag=f"ax{ft}")
                nc.sync.dma_start(
                    out=at[:],
                    in_=xaux[ft * P : (ft + 1) * P, :, c * nt : (c + 1) * nt],
                )
                axs.append(at)
            for gt in range(FT):
                xg = xcs[gt][:]  # [P, K(slots), nt] f16, PI order
                dc = dcpool.tile([P, K, nt], f16, tag="dc")

                # Scalar head: dcB of the PREVIOUS iter (its matmuls are
                # long done -> ScalarE starts the step immediately)
                if prev is not None:
                    nc.scalar.copy(
                        out=prev["dc"][:, KH:K, :], in_=prev["pdB"][:]
                    )
                # ---- matmul halves -> PSUM ----
                pds = []
                for half, pool in ((0, papool), (1, pbpool)):
                    pd = pool.tile([P, KH, nt], f32, tag=("pda", "pdb")[half])
                    # Dummy matmul absorbs the PSUM-slot-release wait
                    # (1-wait limit on Matmult structs).
                    nc.tensor.matmul(
                        pd[:, 0, 0:1], wsb[0][:, 0:P], wsb[0][:, 0:1],
                        start=True, stop=True,
                    )
                    k0 = half * KH
                    for ft in range(FT):
                        for jj in range(KH):
                            nc.tensor.matmul(
                                pd[:, jj, :],
                                wsb[ft][:, gt * P : (gt + 1) * P],
                                xcs[ft][:, k0 + jj, :],
                                start=(ft == 0),
                                stop=(ft == FT - 1),
                            )
                    pds.append(pd)
                    if half == 0:
                        # dcA for THIS iter (right after matmul half A)
                        nc.scalar.copy(out=dc[:, 0:KH, :], in_=pd[:])

                # ---- DVE: elementwise for iter i-1, then flush i-2 ----
                if prev is not None:
                    emit_tail(prev)
                    pending.append(prev)
                    emit_gate(prev)
                if len(pending) == 2:
                    fl = pending.pop(0)
                    flush(fl)
                    outq.append(fl)
                if len(outq) == 1:
                    emit_outs(outq.pop(0))

                prev = {"dc": dc, "xg": xg, "ax": axs[gt][:],
                        "pdB": pds[1], "c": c, "gt": gt}

        # drain: dcB + elementwise + gate for the last iter, then flush all
        nc.scalar.copy(out=prev["dc"][:, KH:K, :], in_=prev["pdB"][:])
        emit_tail(prev)
        pending.append(prev)
        emit_gate(prev)
        for st in outq:
            emit_outs(st)
        for st in pending:
            flush(st)
            emit_outs(st)

    _split_waits(nc)
    return nc


# Engine datapath structs (Matmult/TT/STT/Act/...) only carry ONE sync wait on
# TRN2 walrus; sequencer instructions (NoOp) can each carry one more.  Hoist
# surplus waits onto same-engine NoOps placed just before the instruction.
_SEQ_OK = set()  # every struct on this walrus takes at most ONE sync wait


def _split_waits(nc):
    nnop = 0
    for fn in nc.m.functions:
        for blk in fn.blocks:
            out = []
            for inst in blk.instructions:
                si = inst.sync_info
                if (
                    si is not None
                    and si.on_wait
                    and len(si.on_wait) > 1
                    and type(inst).__name__ not in _SEQ_OK
                ):
                    for w in si.on_wait[:-1]:
                        nop = mybir.InstNoOp(
                            name=f"{inst.name}-sw{nnop}",
                            opcode="NoOp",
                            engine=inst.engine,
                            sync_info=mybir.SyncInfo(on_wait=[w], on_update=[]),
                        )
                        nnop += 1
                        out.append(nop)
                    inst.sync_info = mybir.SyncInfo(
                        on_wait=[si.on_wait[-1]], on_update=list(si.on_update)
                    )
                out.append(inst)
            blk.instructions[:] = out
    return nc


_NC_CACHE = {}


def _get_nc(n_total=N, nt=512):
    key = (n_total, nt)
    if key not in _NC_CACHE:
        _NC_CACHE[key] = build_nc(n_total, nt)
    return _NC_CACHE[key]


def _to_f16(a: np.ndarray) -> np.ndarray:
    return np.ascontiguousarray(a.astype(np.float16))


def _make_xaux(x: np.ndarray) -> np.ndarray:
    # z-planes for k=0,4 of the Killing pairing, computed in fp32 on host
    xa = np.stack(
        [2.0 * x[:, :, 0, :] - x[:, :, 4, :], 2.0 * x[:, :, 4, :] - x[:, :, 0, :]],
        axis=2,
    )
    return _to_f16(xa)


def kernel(x: np.ndarray, W: np.ndarray) -> np.ndarray:
    assert x.shape == (B, F, K, N) and W.shape == (F, F)
    wt = _to_f16(W.T.copy())
    x16 = _to_f16(x)
    xa16 = _make_xaux(x)
    in_maps = [{"x": x16[b], "wt": wt, "xaux": xa16[b]} for b in range(B)]
    nc = _get_nc()
    res = run_bass_kernel_spmd(nc, in_maps, list(range(B)))
    return np.stack(
        [res.results[b]["out"].astype(np.float32) for b in range(B)], axis=0
    )


if __name__ == "__main__":
    xs = np.random.randn(B, F, K, N).astype(np.float32)
    Ws = (np.random.randn(F, F) / np.sqrt(F)).astype(np.float32)
    o = kernel(xs, Ws)
    print(o.shape, o.dtype)
